# revision 10
# baseline (speedup 1.0000x reference)
"""Trainium2 Bass kernel for nn_DQN CEM sampling problem.

Data-parallel over batch: 4096 rows -> 8 cores x 512 rows. Each core runs the
full 99-step CEM loop on its shard; the tiny MLP weights are replicated.

Per-core layout (B=512 rows as G=4 groups of P=128 partitions):
  - MLP runs feature-major: activations [feature, n] with n = (g, p, m) flattened,
    so each layer is a single tensor-engine pass (K on partitions).
  - q is produced batch-major-compatible via 4-way column-tiled matmuls
    (tile_position=(0,32g)), drained to SBUF, then "flipped" (partition<->free)
    with strided SBUF->SBUF DMAs into a [128, 4, 64] tile (cols 50:64 = -1e30 pad).
  - top-32 of 50 per row via a tie-exact bitonic network (sort both 32-halves
    descending, then one compare of half A vs reversed half B) on the vector
    engine; mean/var via bn_stats/bn_aggr; std = sqrt(var * 32/31) (ddof=1).
  - next angles = mu + std * eps sampled batch-major, flipped back into the
    feature-major x row via DMA.

All PRNG tensors (angles0, eps_t) are host-precomputed with the exact jax calls
the reference makes (key 42), laid out per-core, and streamed from DRAM.
"""

import numpy as np

BATCH = 4096
M = 50
NTOP = 32
ITERS = 100  # reference ITERS; device runs ITERS-1 = 99 qnet/stats steps
HIDDEN = 100
NCORES = 8
B = BATCH // NCORES  # 512 rows per core
G = 4                # partition groups per core
P = 128              # rows per group (partitions)
NPG = P * M          # columns per group = 6400
N = G * NPG          # columns per core = 25600
NEG = -1.0e30
TWO_PI = 6.283185307179586

_PROG_CACHE = {}


def _mlp_tiles():
    """(ti, w) matmul column tiles covering one group's NPG columns."""
    tiles = []
    off = 0
    while off < NPG:
        w = min(512, NPG - off)
        tiles.append((off, w))
        off += w
    return tiles


def build_program(n_steps=ITERS - 1):
    """Build the single-core Bass/Tile program (SPMD across cores).

    n_steps = number of qnet+stats iterations (reference does 99)."""
    import concourse.bacc as bacc
    import concourse.bass as bass
    import concourse.tile as tile
    import concourse.mybir as mybir

    f32 = mybir.dt.float32
    f32r = mybir.dt.float32r
    Alu = mybir.AluOpType
    Act = mybir.ActivationFunctionType

    nc = bacc.Bacc("TRN2", target_bir_lowering=False, debug=False)

    # DRAM I/O
    XREP = nc.dram_tensor("XREP", [2, N], f32r, kind="ExternalInput")
    A0 = nc.dram_tensor("A0", [N], f32r, kind="ExternalInput")
    EPS = nc.dram_tensor("EPS", [max(n_steps - 1, 1), P, G * M], f32,
                         kind="ExternalInput")
    W1D = nc.dram_tensor("W1D", [3, HIDDEN], f32r, kind="ExternalInput")
    W2D = nc.dram_tensor("W2D", [HIDDEN, HIDDEN], f32r, kind="ExternalInput")
    W3D = nc.dram_tensor("W3D", [HIDDEN, 64], mybir.dt.float16, kind="ExternalInput")
    B1D = nc.dram_tensor("B1D", [HIDDEN], f32, kind="ExternalInput")
    B2D = nc.dram_tensor("B2D", [HIDDEN], f32, kind="ExternalInput")
    B3D = nc.dram_tensor("B3D", [P], f32, kind="ExternalInput")
    OUT = nc.dram_tensor("OUT", [B], f32, kind="ExternalOutput")

    tiles = _mlp_tiles()

    with tile.TileContext(nc) as tc:
        with (
            tc.tile_pool(name="statics", bufs=1) as statics,
            tc.tile_pool(name="h1pool", bufs=6) as h1pool,
            tc.tile_pool(name="h2pool", bufs=6) as h2pool,
            tc.tile_pool(name="ps1", bufs=3, space=bass.MemorySpace.PSUM) as ps1,
            tc.tile_pool(name="ps2", bufs=3, space=bass.MemorySpace.PSUM) as ps2,
            tc.tile_pool(name="psq", bufs=2, space=bass.MemorySpace.PSUM) as psq,
        ):
            # --- static tiles ---
            x = statics.tile([3, N], f32r)             # fm input rows s0,s1,angle
            q_sb = statics.tile([P, NPG], f32)        # q rows at partitions 0/32/64/96
            q64 = statics.tile([P, G, 64], f32)       # batch-major q + pad
            SA = statics.tile([P, G * 64], f32)       # sort ping
            SB = statics.tile([P, G * 64], f32)       # sort pong
            top32 = statics.tile([P, G, NTOP], f32)
            bnst = statics.tile([P, G, 6], f32)
            mv = statics.tile([P, G, 2], f32)
            mu = statics.tile([P, G], f32)
            std = statics.tile([P, G], f32)
            a_bm = statics.tile([P, G, M], f32r)        # sampled angles, batch-major
            tmp_s = statics.tile([P, G, M], f32)
            eps_sb = statics.tile([P, G * M], f32)
            out_sb = statics.tile([P, G], f32)
            w1s = statics.tile([3, HIDDEN], f32r)
            w2s = statics.tile([HIDDEN, HIDDEN], f32r)
            w3s = statics.tile([HIDDEN, 64], mybir.dt.float16)
            b1s = statics.tile([HIDDEN, 1], f32)
            b2s = statics.tile([HIDDEN, 1], f32)
            b3s = statics.tile([P, 1], f32)

            # --- one-time setup ---
            nc.sync.dma_start(out=w1s, in_=W1D.ap())
            nc.sync.dma_start(out=w2s, in_=W2D.ap())
            nc.sync.dma_start(out=w3s, in_=W3D.ap())
            nc.sync.dma_start(out=b1s, in_=B1D.ap())
            nc.sync.dma_start(out=b2s, in_=B2D.ap())
            nc.sync.dma_start(out=b3s, in_=B3D.ap())
            nc.sync.dma_start(out=x[0:2, :], in_=XREP.ap())
            nc.vector.memset(q64[:, :, M:64], NEG)

            def mlp_and_q(pair):
                """x row2 -> q_sb for the pair's two groups (feature-major MLP).

                Matmuls batched per weight so LDWEIGHTS pipelines under MMs;
                l3 runs 2-way column-tiled (M=64 zero-padded W3) so both
                groups' q rows land in one PSUM tile (partitions 0 and 64)."""
                gs = (2 * pair, 2 * pair + 1)
                for (off, w) in tiles:
                    h1ps, h1ss, h2ss = [], [], []
                    for g in gs:
                        col = g * NPG + off
                        h1p = ps1.tile([HIDDEN, 512], f32, tag="h1p")
                        nc.tensor.matmul(h1p[:, :w], w1s, x[:, col:col + w])
                        h1ps.append(h1p)
                    for j in range(2):
                        h1s = h1pool.tile([HIDDEN, 512], f32r, tag="h1s")
                        nc.scalar.activation(h1s[:, :w], h1ps[j][:, :w],
                                             Act.Relu, bias=b1s, scale=1.0)
                        h1ss.append(h1s)
                    for j in range(2):
                        h2p = ps2.tile([HIDDEN, 512], f32, tag="h2p")
                        nc.tensor.matmul(h2p[:, :w], w2s, h1ss[j][:, :w])
                        h2s = h2pool.tile([HIDDEN, 512], mybir.dt.float16,
                                          tag="h2s")
                        nc.vector.tensor_scalar(h2s[:, :w], h2p[:, :w],
                                                scalar1=b2s, scalar2=0.0,
                                                op0=Alu.add, op1=Alu.max)
                        h2ss.append(h2s)
                    qp = psq.tile([P, 512], f32, tag="qp")
                    for j in range(2):
                        nc.tensor.matmul(qp[64 * j:64 * j + 64, :w], w3s,
                                         h2ss[j][:, :w],
                                         tile_position=(0, 64 * j))
                    # drain both groups' q rows (plus zero rows) in one op
                    nc.scalar.copy(q_sb[:, off:off + w], qp[:, :w])

            def q_flip(pair):
                for j in range(2):
                    g = 2 * pair + j
                    nc.sync.dma_start(
                        out=q64[:, g, 0:M],
                        in_=q_sb[64 * j:64 * j + 1, :].rearrange(
                            "a (p m) -> a p m", m=M),
                    )

            def a_flip(pair):
                for j in range(2):
                    g = 2 * pair + j
                    nc.sync.dma_start(
                        out=x[2:3, g * NPG:(g + 1) * NPG].rearrange(
                            "a (p m) -> a p m", m=M),
                        in_=a_bm[:, g, :],
                    )

            def sort_stats(pair):
                """q64[pair groups] -> top32 -> mv (mean, var) -> mu."""
                g0 = 2 * pair

                def hv(t):
                    # pair slice as [P, 4 halves, 32]
                    return t[:, pair * 128:(pair + 1) * 128].rearrange(
                        "p (h m) -> p h m", m=32)

                src = hv(q64.rearrange("p g m -> p (g m)"))
                dst_list = [hv(SA), hv(SB)]
                which = 0
                for k in [2, 4, 8, 16, 32]:
                    dst = dst_list[which]; which ^= 1
                    s4 = src.rearrange("p h (nb k) -> p h nb k", k=k)
                    d4 = dst.rearrange("p h (nb k) -> p h nb k", k=k)
                    nc.vector.tensor_tensor(d4[:, :, :, 0:k // 2],
                                            s4[:, :, :, 0:k // 2],
                                            s4[:, :, :, k - 1:k // 2 - 1:-1],
                                            op=Alu.max)
                    nc.vector.tensor_tensor(d4[:, :, :, k // 2:k],
                                            s4[:, :, :, k // 2:k],
                                            s4[:, :, :, k // 2 - 1::-1],
                                            op=Alu.min)
                    src = dst
                    d = k // 4
                    while d >= 1:
                        dst = dst_list[which]; which ^= 1
                        s5 = src.rearrange("p h (nb two d) -> p h nb two d",
                                           two=2, d=d)
                        d5 = dst.rearrange("p h (nb two d) -> p h nb two d",
                                           two=2, d=d)
                        nc.vector.tensor_tensor(d5[:, :, :, 0, :],
                                                s5[:, :, :, 0, :],
                                                s5[:, :, :, 1, :], op=Alu.max)
                        nc.vector.tensor_tensor(d5[:, :, :, 1, :],
                                                s5[:, :, :, 0, :],
                                                s5[:, :, :, 1, :], op=Alu.min)
                        src = dst
                        d //= 2
                # merge: top32[p,g,i] = max(A[p,g,i], B[p,g,31-i])
                sg = src.rearrange("p (g h) m -> p g h m", g=2)
                nc.vector.tensor_tensor(top32[:, g0:g0 + 2, :],
                                        sg[:, :, 0, :],
                                        sg[:, :, 1, ::-1], op=Alu.max)
                for g in (g0, g0 + 1):
                    nc.vector.bn_stats(bnst[:, g, :], top32[:, g, :])
                    nc.vector.bn_aggr(mv[:, g, :], bnst[:, g:g + 1, :])
                nc.vector.tensor_scalar(mu[:, g0:g0 + 2], mv[:, g0:g0 + 2, 0],
                                        scalar1=b3s, scalar2=None, op0=Alu.add)

            def sample(pair):
                """a_bm[pair] = mu + std * eps (eps_sb already loaded)."""
                g0 = 2 * pair
                nc.scalar.activation(std[:, g0:g0 + 2], mv[:, g0:g0 + 2, 1],
                                     Act.Sqrt, scale=float(NTOP) / (NTOP - 1))
                epsv = eps_sb.rearrange("p (g m) -> p g m", m=M)
                stdb = std[:, g0:g0 + 2].unsqueeze(2).to_broadcast((P, 2, M))
                mub = mu[:, g0:g0 + 2].unsqueeze(2).to_broadcast((P, 2, M))
                nc.vector.tensor_tensor(tmp_s[:, g0:g0 + 2, :],
                                        epsv[:, g0:g0 + 2, :], stdb,
                                        op=Alu.mult)
                nc.vector.tensor_tensor(a_bm[:, g0:g0 + 2, :],
                                        tmp_s[:, g0:g0 + 2, :], mub,
                                        op=Alu.add)

            def load_eps(t):
                if isinstance(t, int):
                    eps_src = EPS.ap()[t:t + 1, :, :]
                else:
                    eps_src = EPS.ap()[bass.ds(t, 1), :, :]
                nc.sync.dma_start(out=eps_sb, in_=eps_src)

            def iteration(t, first, last):
                if not last:
                    load_eps(t)
                for pair in range(2):
                    if not first:
                        a_flip(pair)
                    mlp_and_q(pair)
                    q_flip(pair)
                    sort_stats(pair)
                    if not last:
                        sample(pair)
                # a_flips for the next iteration happen at its start (the
                # write of x row2 must come after this iteration's l1 reads,
                # which the tile dependency tracker orders via the pair split)

            # ---- t = 0 ----
            nc.sync.dma_start(out=x[2:3, :],
                              in_=A0.ap().rearrange("(a n) -> a n", a=1))
            iteration(0, True, n_steps == 1)

            # ---- t = 1 .. n_steps-2 (dynamic loop) ----
            if n_steps > 2:
                with tc.For_i(1, n_steps - 1,
                              hint_engines=(mybir.EngineType.PE,)) as it:
                    iteration(it, False, False)

            # ---- t = n_steps-1 (final) ----
            if n_steps > 1:
                iteration(n_steps - 1, False, True)
            nc.vector.tensor_scalar(out_sb, mu, scalar1=TWO_PI, scalar2=None,
                                    op0=Alu.mult)
            nc.sync.dma_start(out=OUT.ap().rearrange("(g p) -> p g", p=P),
                              in_=out_sb)

    nc.compile()
    return nc


def host_prng(n_steps=ITERS - 1):
    """Exactly the reference's PRNG stream, on host CPU."""
    import jax
    import jax.numpy as jnp
    cpu = jax.devices("cpu")[0]
    with jax.default_device(cpu):
        # commit the key to CPU so the whole PRNG stream is computed by the
        # CPU backend bit-exactly (matching the reference harness)
        key = jax.device_put(jax.random.key(42), cpu)
        k0, kloop = jax.random.split(key)
        angles0 = np.asarray(jax.random.uniform(k0, (BATCH, M),
                                                dtype=jnp.float32))
        keys = jax.random.split(kloop, ITERS - 1)
        eps = np.stack([
            np.asarray(jax.random.normal(keys[t], (BATCH, M),
                                         dtype=jnp.float32))
            for t in range(max(n_steps - 1, 1))
        ])
    return angles0, eps


def make_in_map(core, states, W1, b1, W2, b2, W3, b3, angles0, eps):
    sl = slice(core * B, (core + 1) * B)
    S = np.ascontiguousarray(states[sl]).reshape(G, P, 2)
    xrep = np.ascontiguousarray(
        np.broadcast_to(S[:, :, None, :], (G, P, M, 2)).transpose(3, 0, 1, 2)
    ).reshape(2, N)
    a0 = np.ascontiguousarray(angles0[sl]).reshape(N)
    epsc = np.ascontiguousarray(
        eps[:, sl, :].reshape(-1, G, P, M).transpose(0, 2, 1, 3)
    ).reshape(-1, P, G * M)
    w3p = np.zeros((HIDDEN, 64), np.float16)
    w3p[:, 0] = W3[:, 0].astype(np.float16)
    return {
        "XREP": xrep.astype(np.float32),
        "A0": a0.astype(np.float32),
        "EPS": epsc.astype(np.float32),
        "W1D": W1.astype(np.float32),
        "W2D": W2.astype(np.float32),
        "W3D": w3p,  # fp16
        "B1D": b1.astype(np.float32),
        "B2D": b2.astype(np.float32),
        "B3D": np.full((P,), np.float32(b3[0]), np.float32),
    }


LAST_RESULTS = None


def kernel(states, W1, b1, W2, b2, W3, b3, _trace=False):
    global LAST_RESULTS
    from concourse.bass_utils import run_bass_kernel_spmd

    n_steps = ITERS - 1
    if n_steps not in _PROG_CACHE:
        _PROG_CACHE[n_steps] = build_program(n_steps)
    nc = _PROG_CACHE[n_steps]

    angles0, eps = host_prng(n_steps)
    in_maps = [
        make_in_map(c, states, W1, b1, W2, b2, W3, b3, angles0, eps)
        for c in range(NCORES)
    ]
    res = run_bass_kernel_spmd(nc, in_maps, core_ids=list(range(NCORES)),
                               trace=_trace)
    LAST_RESULTS = res
    out = np.concatenate([res.results[c]["OUT"] for c in range(NCORES)])
    return out.astype(np.float32)


# revision 11
# speedup vs baseline: 1.1854x; 1.1854x over previous
"""Trainium2 Bass kernel for nn_DQN CEM sampling problem.

Data-parallel over batch: 4096 rows -> 8 cores x 512 rows. Each core runs the
full 99-step CEM loop on its shard; the tiny MLP weights are replicated.

Per-core layout (B=512 rows as two PAIRS of groups, each group P=128 rows):
  - MLP runs feature-major: activations [feature, n], n = (g, p, m) flattened;
    l1/l2 use fp32r (full-rate 4-byte matmul), l3 uses fp16 so its output can
    be 2-way column-tiled (zero-padded W3 to M=64) with both groups' q rows
    landing in one PSUM tile (partitions 0 and 64).
  - q is drained to SBUF then "flipped" (partition<->free) with strided
    SBUF->SBUF DMAs into a [128, g, 64] tile (cols 50:64 = -1e30 pad).
  - top-32 of 50 per row via a tie-exact bitonic network on the vector engine;
    mean/var via bn_stats/bn_aggr; std = sqrt(var * 32/31) (ddof=1).
  - next angles = mu + std * eps sampled batch-major, flipped back into the
    feature-major x row via DMA.
  - software pipeline: each phase runs one pair's MLP on the tensor engine
    while the OTHER pair's top-k/stats/sampling (from the previous phase) is
    interleaved into the vector/scalar/DMA streams, so the PE never idles and
    its clock governor stays warm. The pipeline is carried across the dynamic
    loop's back-edge (pair B's tail runs at the start of the next iteration).

All PRNG tensors (angles0, eps_t) are host-precomputed with the exact jax calls
the reference makes (key 42), laid out per-core, and streamed from DRAM.
"""

import numpy as np

BATCH = 4096
M = 50
NTOP = 32
ITERS = 100  # reference ITERS; device runs ITERS-1 = 99 qnet/stats steps
HIDDEN = 100
NCORES = 8
B = BATCH // NCORES  # 512 rows per core
G = 4                # partition groups per core
P = 128              # rows per group (partitions)
NPG = P * M          # columns per group = 6400
N = G * NPG          # columns per core = 25600
NEG = -1.0e30
TWO_PI = 6.283185307179586

_PROG_CACHE = {}


def _mlp_tiles():
    tiles = []
    off = 0
    while off < NPG:
        w = min(512, NPG - off)
        tiles.append((off, w))
        off += w
    return tiles


def build_program(n_steps=ITERS - 1):
    """Build the single-core Bass/Tile program (SPMD across cores)."""
    import concourse.bacc as bacc
    import concourse.bass as bass
    import concourse.tile as tile
    import concourse.mybir as mybir

    f32 = mybir.dt.float32
    f32r = mybir.dt.float32r
    fp16 = mybir.dt.float16
    Alu = mybir.AluOpType
    Act = mybir.ActivationFunctionType

    nc = bacc.Bacc("TRN2", target_bir_lowering=False, debug=False)

    XREP = nc.dram_tensor("XREP", [2, N], f32r, kind="ExternalInput")
    A0 = nc.dram_tensor("A0", [N], f32r, kind="ExternalInput")
    EPS = nc.dram_tensor("EPS", [max(n_steps - 1, 1), P, G * M], f32,
                         kind="ExternalInput")
    W1D = nc.dram_tensor("W1D", [3, HIDDEN], f32r, kind="ExternalInput")
    W2D = nc.dram_tensor("W2D", [HIDDEN, HIDDEN], f32r, kind="ExternalInput")
    W3D = nc.dram_tensor("W3D", [HIDDEN, 64], fp16, kind="ExternalInput")
    B1D = nc.dram_tensor("B1D", [HIDDEN], f32, kind="ExternalInput")
    B2D = nc.dram_tensor("B2D", [HIDDEN], f32, kind="ExternalInput")
    B3D = nc.dram_tensor("B3D", [P], f32, kind="ExternalInput")
    OUT = nc.dram_tensor("OUT", [B], f32, kind="ExternalOutput")

    tiles = _mlp_tiles()

    with tile.TileContext(nc) as tc:
        with (
            tc.tile_pool(name="statics", bufs=1) as statics,
            tc.tile_pool(name="h1pool", bufs=6) as h1pool,
            tc.tile_pool(name="h2pool", bufs=6) as h2pool,
            tc.tile_pool(name="ps1", bufs=3, space=bass.MemorySpace.PSUM) as ps1,
            tc.tile_pool(name="ps2", bufs=3, space=bass.MemorySpace.PSUM) as ps2,
            tc.tile_pool(name="psq", bufs=2, space=bass.MemorySpace.PSUM) as psq,
        ):
            # --- static tiles ---
            x = statics.tile([3, N], f32r)      # fm input rows s0, s1, angle
            q_sbA = statics.tile([P, NPG], f32)  # pair A q rows (parts 0/64)
            q_sbB = statics.tile([P, NPG], f32)  # pair B q rows
            q64 = statics.tile([P, G, 64], f32)  # batch-major q + pad
            SA = statics.tile([P, G * 64], f32)
            SB = statics.tile([P, G * 64], f32)
            top32 = statics.tile([P, G, NTOP], f32)
            bnst = statics.tile([P, G, 6], f32)
            mv = statics.tile([P, G, 2], f32)
            mu = statics.tile([P, G], f32)
            std = statics.tile([P, G], f32)
            a_bm = statics.tile([P, G, M], f32r)
            tmp_s = statics.tile([P, G, M], f32)
            eps_sbA = statics.tile([P, 2 * M], f32)
            eps_sbB = statics.tile([P, 2 * M], f32)
            out_sb = statics.tile([P, G], f32)
            w1s = statics.tile([3, HIDDEN], f32r)
            w2s = statics.tile([HIDDEN, HIDDEN], f32r)
            w3s = statics.tile([HIDDEN, 64], fp16)
            b1s = statics.tile([HIDDEN, 1], f32)
            b2s = statics.tile([HIDDEN, 1], f32)
            b3s = statics.tile([P, 1], f32)

            q_sbs = (q_sbA, q_sbB)
            eps_sbs = (eps_sbA, eps_sbB)

            # --- one-time setup ---
            nc.sync.dma_start(out=w1s, in_=W1D.ap())
            nc.sync.dma_start(out=w2s, in_=W2D.ap())
            nc.sync.dma_start(out=w3s, in_=W3D.ap())
            nc.sync.dma_start(out=b1s, in_=B1D.ap())
            nc.sync.dma_start(out=b2s, in_=B2D.ap())
            nc.sync.dma_start(out=b3s, in_=B3D.ap())
            nc.sync.dma_start(out=x[0:2, :], in_=XREP.ap())
            nc.vector.memset(q64[:, :, M:64], NEG)

            def load_eps(pair, t):
                """eps half-row t for this pair -> its eps_sb tile."""
                lo, hi = (0, 2 * M) if pair == 0 else (2 * M, 4 * M)
                if isinstance(t, int):
                    src = EPS.ap()[t:t + 1, :, lo:hi]
                else:
                    src = EPS.ap()[bass.ds(t, 1), :, lo:hi]
                nc.sync.dma_start(out=eps_sbs[pair], in_=src)

            def mlp_tile_gen(pair):
                """Generator: emits one column-tile of the pair's MLP per
                next() call. Matmuls batched per weight so LDWEIGHTS
                pipelines under the previous MM."""
                gs = (2 * pair, 2 * pair + 1)
                qsb = q_sbs[pair]
                for (off, w) in tiles:
                    h1ps, h1ss, h2ss = [], [], []
                    for g in gs:
                        col = g * NPG + off
                        h1p = ps1.tile([HIDDEN, 512], f32, tag="h1p")
                        nc.tensor.matmul(h1p[:, :w], w1s, x[:, col:col + w])
                        h1ps.append(h1p)
                    for j in range(2):
                        h1s = h1pool.tile([HIDDEN, 512], f32r, tag="h1s")
                        nc.scalar.activation(h1s[:, :w], h1ps[j][:, :w],
                                             Act.Relu, bias=b1s, scale=1.0)
                        h1ss.append(h1s)
                    for j in range(2):
                        h2p = ps2.tile([HIDDEN, 512], f32, tag="h2p")
                        nc.tensor.matmul(h2p[:, :w], w2s, h1ss[j][:, :w])
                        h2s = h2pool.tile([HIDDEN, 512], fp16, tag="h2s")
                        nc.vector.tensor_scalar(h2s[:, :w], h2p[:, :w],
                                                scalar1=b2s, scalar2=0.0,
                                                op0=Alu.add, op1=Alu.max)
                        h2ss.append(h2s)
                    qp = psq.tile([P, 512], f32, tag="qp")
                    for j in range(2):
                        nc.tensor.matmul(qp[64 * j:64 * j + 64, :w], w3s,
                                         h2ss[j][:, :w],
                                         tile_position=(0, 64 * j))
                    nc.scalar.copy(qsb[:, off:off + w], qp[:, :w])
                    yield

            def tail_thunks(pair, do_sample):
                """Top-k/stats/sample for this pair's q (already in q_sb) as a
                list of single-instruction thunks, in dependency order."""
                g0 = 2 * pair
                qsb = q_sbs[pair]
                ths = []

                for j in range(2):
                    def qflip(j=j):
                        nc.sync.dma_start(
                            out=q64[:, g0 + j, 0:M],
                            in_=qsb[64 * j:64 * j + 1, :].rearrange(
                                "a (p m) -> a p m", m=M))
                    ths.append(qflip)

                def hv(t):
                    return t[:, pair * 128:(pair + 1) * 128].rearrange(
                        "p (h m) -> p h m", m=32)

                src = hv(q64.rearrange("p g m -> p (g m)"))
                dst_list = [hv(SA), hv(SB)]
                which = 0
                for k in [2, 4, 8, 16, 32]:
                    dst = dst_list[which]; which ^= 1
                    s4 = src.rearrange("p h (nb k) -> p h nb k", k=k)
                    d4 = dst.rearrange("p h (nb k) -> p h nb k", k=k)

                    def flip_max(d4=d4, s4=s4, k=k):
                        nc.vector.tensor_tensor(
                            d4[:, :, :, 0:k // 2], s4[:, :, :, 0:k // 2],
                            s4[:, :, :, k - 1:k // 2 - 1:-1], op=Alu.max)

                    def flip_min(d4=d4, s4=s4, k=k):
                        nc.vector.tensor_tensor(
                            d4[:, :, :, k // 2:k], s4[:, :, :, k // 2:k],
                            s4[:, :, :, k // 2 - 1::-1], op=Alu.min)
                    ths += [flip_max, flip_min]
                    src = dst
                    d = k // 4
                    while d >= 1:
                        dst = dst_list[which]; which ^= 1
                        s5 = src.rearrange("p h (nb two d) -> p h nb two d",
                                           two=2, d=d)
                        d5 = dst.rearrange("p h (nb two d) -> p h nb two d",
                                           two=2, d=d)

                        def plain_max(d5=d5, s5=s5):
                            nc.vector.tensor_tensor(
                                d5[:, :, :, 0, :], s5[:, :, :, 0, :],
                                s5[:, :, :, 1, :], op=Alu.max)

                        def plain_min(d5=d5, s5=s5):
                            nc.vector.tensor_tensor(
                                d5[:, :, :, 1, :], s5[:, :, :, 0, :],
                                s5[:, :, :, 1, :], op=Alu.min)
                        ths += [plain_max, plain_min]
                        src = dst
                        d //= 2

                sg = src.rearrange("p (g h) m -> p g h m", g=2)

                def merge(sg=sg):
                    nc.vector.tensor_tensor(top32[:, g0:g0 + 2, :],
                                            sg[:, :, 0, :],
                                            sg[:, :, 1, ::-1], op=Alu.max)
                ths.append(merge)

                for g in (g0, g0 + 1):
                    def bns(g=g):
                        nc.vector.bn_stats(bnst[:, g, :], top32[:, g, :])

                    def bna(g=g):
                        nc.vector.bn_aggr(mv[:, g, :], bnst[:, g:g + 1, :])
                    ths += [bns, bna]

                def mu_op():
                    nc.vector.tensor_scalar(mu[:, g0:g0 + 2],
                                            mv[:, g0:g0 + 2, 0],
                                            scalar1=b3s, scalar2=None,
                                            op0=Alu.add)
                ths.append(mu_op)

                if do_sample:
                    def sqrt_op():
                        nc.scalar.activation(std[:, g0:g0 + 2],
                                             mv[:, g0:g0 + 2, 1], Act.Sqrt,
                                             scale=float(NTOP) / (NTOP - 1))
                    ths.append(sqrt_op)

                    epsv = eps_sbs[pair].rearrange("p (g m) -> p g m", m=M)
                    stdb = std[:, g0:g0 + 2].unsqueeze(2).to_broadcast(
                        (P, 2, M))
                    mub = mu[:, g0:g0 + 2].unsqueeze(2).to_broadcast((P, 2, M))

                    def smul(epsv=epsv, stdb=stdb):
                        nc.vector.tensor_tensor(tmp_s[:, g0:g0 + 2, :], epsv,
                                                stdb, op=Alu.mult)
                    ths.append(smul)

                    def sadd(mub=mub):
                        nc.vector.tensor_tensor(a_bm[:, g0:g0 + 2, :],
                                                tmp_s[:, g0:g0 + 2, :], mub,
                                                op=Alu.add)
                    ths.append(sadd)

                    for j in range(2):
                        def aflip(j=j):
                            g = g0 + j
                            nc.sync.dma_start(
                                out=x[2:3, g * NPG:(g + 1) * NPG].rearrange(
                                    "a (p m) -> a p m", m=M),
                                in_=a_bm[:, g, :])
                        ths.append(aflip)
                return ths

            def phase(mlp_pair, tail):
                """Emit one pair's MLP with the other pair's tail interleaved
                into the vector/scalar/DMA instruction streams."""
                thunks = tail_thunks(*tail) if tail is not None else []
                if mlp_pair is None:
                    for th in thunks:
                        th()
                    return
                gen = mlp_tile_gen(mlp_pair)
                nt = len(tiles)
                per = (len(thunks) + nt - 1) // nt if thunks else 0
                ti = 0
                for _ in gen:
                    ti += 1
                    lo = per * (ti - 1)
                    for th in thunks[lo:lo + per]:
                        th()
                for th in thunks[per * ti:]:
                    th()

            # ---- prologue: t = 0 ----
            nc.sync.dma_start(out=x[2:3, :],
                              in_=A0.ap().rearrange("(a n) -> a n", a=1))
            phase(0, None)
            if n_steps > 1:
                load_eps(0, 0)
            phase(1, (0, n_steps > 1))

            # ---- t = 1 .. n_steps-2: pipelined loop ----
            if n_steps > 2:
                with tc.For_i(1, n_steps - 1,
                              hint_engines=(mybir.EngineType.PE,)) as it:
                    load_eps(1, it - 1)
                    load_eps(0, it)
                    phase(0, (1, True))   # pair B tail from iteration it-1
                    phase(1, (0, True))   # pair A tail of iteration it

            # ---- epilogue: t = n_steps-1 ----
            if n_steps > 1:
                load_eps(1, n_steps - 2)
                phase(0, (1, True))       # pair B tail from t = n_steps-2
                phase(1, (0, False))      # pair A final stats
            phase(None, (1, False))       # pair B final stats
            nc.vector.tensor_scalar(out_sb, mu, scalar1=TWO_PI, scalar2=None,
                                    op0=Alu.mult)
            nc.sync.dma_start(out=OUT.ap().rearrange("(g p) -> p g", p=P),
                              in_=out_sb)

    nc.compile()
    return nc


def host_prng(n_steps=ITERS - 1):
    """Exactly the reference's PRNG stream, on host CPU."""
    import jax
    import jax.numpy as jnp
    cpu = jax.devices("cpu")[0]
    with jax.default_device(cpu):
        # commit the key to CPU so the whole PRNG stream is computed by the
        # CPU backend bit-exactly (matching the reference harness)
        key = jax.device_put(jax.random.key(42), cpu)
        k0, kloop = jax.random.split(key)
        angles0 = np.asarray(jax.random.uniform(k0, (BATCH, M),
                                                dtype=jnp.float32))
        keys = jax.random.split(kloop, ITERS - 1)
        eps = np.stack([
            np.asarray(jax.random.normal(keys[t], (BATCH, M),
                                         dtype=jnp.float32))
            for t in range(max(n_steps - 1, 1))
        ])
    return angles0, eps


def make_in_map(core, states, W1, b1, W2, b2, W3, b3, angles0, eps):
    sl = slice(core * B, (core + 1) * B)
    S = np.ascontiguousarray(states[sl]).reshape(G, P, 2)
    xrep = np.ascontiguousarray(
        np.broadcast_to(S[:, :, None, :], (G, P, M, 2)).transpose(3, 0, 1, 2)
    ).reshape(2, N)
    a0 = np.ascontiguousarray(angles0[sl]).reshape(N)
    epsc = np.ascontiguousarray(
        eps[:, sl, :].reshape(-1, G, P, M).transpose(0, 2, 1, 3)
    ).reshape(-1, P, G * M)
    w3p = np.zeros((HIDDEN, 64), np.float16)
    w3p[:, 0] = W3[:, 0].astype(np.float16)
    return {
        "XREP": xrep.astype(np.float32),
        "A0": a0.astype(np.float32),
        "EPS": epsc.astype(np.float32),
        "W1D": W1.astype(np.float32),
        "W2D": W2.astype(np.float32),
        "W3D": w3p,
        "B1D": b1.astype(np.float32),
        "B2D": b2.astype(np.float32),
        "B3D": np.full((P,), np.float32(b3[0]), np.float32),
    }


LAST_RESULTS = None


def kernel(states, W1, b1, W2, b2, W3, b3, _trace=False):
    global LAST_RESULTS
    from concourse.bass_utils import run_bass_kernel_spmd

    n_steps = ITERS - 1
    if n_steps not in _PROG_CACHE:
        _PROG_CACHE[n_steps] = build_program(n_steps)
    nc = _PROG_CACHE[n_steps]

    angles0, eps = host_prng(n_steps)
    in_maps = [
        make_in_map(c, states, W1, b1, W2, b2, W3, b3, angles0, eps)
        for c in range(NCORES)
    ]
    res = run_bass_kernel_spmd(nc, in_maps, core_ids=list(range(NCORES)),
                               trace=_trace)
    LAST_RESULTS = res
    out = np.concatenate([res.results[c]["OUT"] for c in range(NCORES)])
    return out.astype(np.float32)


# revision 14
# speedup vs baseline: 1.2508x; 1.0552x over previous
"""Trainium2 Bass kernel for nn_DQN CEM sampling problem.

Data-parallel over batch: 4096 rows -> 8 cores x 512 rows. Each core runs the
full 99-step CEM loop on its shard; the tiny MLP weights are replicated.

Per-core layout (B=512 rows as two PAIRS of groups, each group P=128 rows):
  - MLP runs feature-major: activations [feature, n], n = (g, p, m) flattened;
    l1/l2 use fp32r (full-rate 4-byte matmul), l3 uses fp16 so its output can
    be 2-way column-tiled (zero-padded W3 to M=64) with both groups' q rows
    landing in one PSUM tile (partitions 0 and 64).
  - q is drained to SBUF then "flipped" (partition<->free) with strided
    SBUF->SBUF DMAs into a [128, g, 64] tile (cols 50:64 = -1e30 pad).
  - top-32 of 50 per row via a tie-exact bitonic network on the vector engine;
    mean/var via bn_stats/bn_aggr; std = sqrt(var * 32/31) (ddof=1).
  - next angles = mu + std * eps sampled batch-major, flipped back into the
    feature-major x row via DMA.
  - software pipeline: each phase runs one pair's MLP on the tensor engine
    while the OTHER pair's top-k/stats/sampling (from the previous phase) is
    interleaved into the vector/scalar/DMA streams, so the PE never idles and
    its clock governor stays warm. The pipeline is carried across the dynamic
    loop's back-edge (pair B's tail runs at the start of the next iteration).

All PRNG tensors (angles0, eps_t) are host-precomputed with the exact jax calls
the reference makes (key 42), laid out per-core, and streamed from DRAM.
"""

import numpy as np

BATCH = 4096
M = 50
NTOP = 32
ITERS = 100  # reference ITERS; device runs ITERS-1 = 99 qnet/stats steps
HIDDEN = 100
NCORES = 8
B = BATCH // NCORES  # 512 rows per core
G = 4                # partition groups per core
P = 128              # rows per group (partitions)
NPG = P * M          # columns per group = 6400
N = G * NPG          # columns per core = 25600
NEG = -1.0e30
TWO_PI = 6.283185307179586

_PROG_CACHE = {}


def _mlp_tiles():
    tiles = []
    off = 0
    while off < NPG:
        w = min(512, NPG - off)
        tiles.append((off, w))
        off += w
    return tiles


def build_program(n_steps=ITERS - 1):
    """Build the single-core Bass/Tile program (SPMD across cores)."""
    import concourse.bacc as bacc
    import concourse.bass as bass
    import concourse.tile as tile
    import concourse.mybir as mybir

    f32 = mybir.dt.float32
    f32r = mybir.dt.float32r
    fp16 = mybir.dt.float16
    Alu = mybir.AluOpType
    Act = mybir.ActivationFunctionType

    nc = bacc.Bacc("TRN2", target_bir_lowering=False, debug=False)

    XREP = nc.dram_tensor("XREP", [2, N], f32r, kind="ExternalInput")
    A0 = nc.dram_tensor("A0", [N], f32r, kind="ExternalInput")
    EPS = nc.dram_tensor("EPS", [max(n_steps - 1, 1), P, G * M], f32,
                         kind="ExternalInput")
    W1D = nc.dram_tensor("W1D", [3, HIDDEN], f32r, kind="ExternalInput")
    W2D = nc.dram_tensor("W2D", [HIDDEN, HIDDEN], f32r, kind="ExternalInput")
    W3D = nc.dram_tensor("W3D", [HIDDEN, 64], fp16, kind="ExternalInput")
    B1D = nc.dram_tensor("B1D", [HIDDEN], f32, kind="ExternalInput")
    B2D = nc.dram_tensor("B2D", [HIDDEN], f32, kind="ExternalInput")
    B3D = nc.dram_tensor("B3D", [P], f32, kind="ExternalInput")
    OUT = nc.dram_tensor("OUT", [B], f32, kind="ExternalOutput")

    tiles = _mlp_tiles()

    with tile.TileContext(nc) as tc:
        with (
            tc.tile_pool(name="statics", bufs=1) as statics,
            tc.tile_pool(name="h1pool", bufs=6) as h1pool,
            tc.tile_pool(name="h2pool", bufs=28) as h2pool,
            tc.tile_pool(name="ps1", bufs=3, space=bass.MemorySpace.PSUM) as ps1,
            tc.tile_pool(name="ps2", bufs=3, space=bass.MemorySpace.PSUM) as ps2,
            tc.tile_pool(name="psq", bufs=2, space=bass.MemorySpace.PSUM) as psq,
        ):
            # --- static tiles ---
            x = statics.tile([3, N], f32r)      # fm input rows s0, s1, angle
            q_sbA = statics.tile([P, NPG], f32)  # pair A q rows (parts 0/64)
            q_sbB = statics.tile([P, NPG], f32)  # pair B q rows
            q64 = statics.tile([P, G, 64], f32)  # batch-major q + pad
            SA = statics.tile([P, G * 64], f32)
            SB = statics.tile([P, G * 64], f32)
            top32 = statics.tile([P, G, NTOP], f32)
            bnst = statics.tile([P, G, 6], f32)
            mv = statics.tile([P, G, 2], f32)
            mu = statics.tile([P, G], f32)
            std = statics.tile([P, G], f32)
            a_bm = statics.tile([P, G, M], f32r)
            tmp_s = statics.tile([P, G, M], f32)
            eps_sbA = statics.tile([P, 2 * M], f32)
            eps_sbB = statics.tile([P, 2 * M], f32)
            out_sb = statics.tile([P, G], f32)
            w1s = statics.tile([3, HIDDEN], f32r)
            w2s = statics.tile([HIDDEN, HIDDEN], f32r)
            w3s = statics.tile([HIDDEN, 64], fp16)
            b1s = statics.tile([HIDDEN, 1], f32)
            b2s = statics.tile([HIDDEN, 1], f32)
            b3s = statics.tile([P, 1], f32)

            q_sbs = (q_sbA, q_sbB)
            eps_sbs = (eps_sbA, eps_sbB)

            # --- one-time setup ---
            nc.sync.dma_start(out=w1s, in_=W1D.ap())
            nc.sync.dma_start(out=w2s, in_=W2D.ap())
            nc.sync.dma_start(out=w3s, in_=W3D.ap())
            nc.sync.dma_start(out=b1s, in_=B1D.ap())
            nc.sync.dma_start(out=b2s, in_=B2D.ap())
            nc.sync.dma_start(out=b3s, in_=B3D.ap())
            nc.sync.dma_start(out=x[0:2, :], in_=XREP.ap())
            nc.vector.memset(q64[:, :, M:64], NEG)

            def load_eps(pair, t):
                """eps half-row t for this pair -> its eps_sb tile."""
                lo, hi = (0, 2 * M) if pair == 0 else (2 * M, 4 * M)
                if isinstance(t, int):
                    src = EPS.ap()[t:t + 1, :, lo:hi]
                else:
                    src = EPS.ap()[bass.ds(t, 1), :, lo:hi]
                nc.sync.dma_start(out=eps_sbs[pair], in_=src)

            def mlp_tile_gen(pair):
                """Generator: emits the pair's MLP, yielding once per
                column-tile. All fp32r matmuls (l1+l2) are emitted first,
                then all fp16 l3 matmuls, so the PE switches fp32 mode only
                twice per phase (mode flips reset the PE clock governor)."""
                gs = (2 * pair, 2 * pair + 1)
                qsb = q_sbs[pair]
                h2_all = []
                for (off, w) in tiles:
                    h1ps, h1ss = [], []
                    for g in gs:
                        col = g * NPG + off
                        h1p = ps1.tile([HIDDEN, 512], f32, tag="h1p")
                        nc.tensor.matmul(h1p[:, :w], w1s, x[:, col:col + w])
                        h1ps.append(h1p)
                    for j in range(2):
                        h1s = h1pool.tile([HIDDEN, 512], f32r, tag="h1s")
                        nc.scalar.activation(h1s[:, :w], h1ps[j][:, :w],
                                             Act.Relu, bias=b1s, scale=1.0)
                        h1ss.append(h1s)
                    h2ss = []
                    for j in range(2):
                        h2p = ps2.tile([HIDDEN, 512], f32, tag="h2p")
                        nc.tensor.matmul(h2p[:, :w], w2s, h1ss[j][:, :w])
                        h2s = h2pool.tile([HIDDEN, 512], fp16, tag="h2s")
                        nc.vector.tensor_scalar(h2s[:, :w], h2p[:, :w],
                                                scalar1=b2s, scalar2=0.0,
                                                op0=Alu.add, op1=Alu.max)
                        h2ss.append(h2s)
                    h2_all.append(h2ss)
                    yield
                for ti, (off, w) in enumerate(tiles):
                    qp = psq.tile([P, 512], f32, tag="qp")
                    for j in range(2):
                        nc.tensor.matmul(qp[64 * j:64 * j + 64, :w], w3s,
                                         h2_all[ti][j][:, :w],
                                         tile_position=(0, 64 * j))
                    nc.scalar.copy(qsb[:, off:off + w], qp[:, :w])
                    yield

            def tail_thunks(pair, do_sample):
                """Top-k/stats/sample for this pair's q (already in q_sb) as a
                list of single-instruction thunks, in dependency order."""
                g0 = 2 * pair
                qsb = q_sbs[pair]
                ths = []

                for j in range(2):
                    def qflip(j=j):
                        nc.sync.dma_start(
                            out=q64[:, g0 + j, 0:M],
                            in_=qsb[64 * j:64 * j + 1, :].rearrange(
                                "a (p m) -> a p m", m=M))
                    ths.append(qflip)

                def hv(t):
                    return t[:, pair * 128:(pair + 1) * 128].rearrange(
                        "p (h m) -> p h m", m=32)

                src = hv(q64.rearrange("p g m -> p (g m)"))
                dst_list = [hv(SA), hv(SB)]
                which = 0
                for k in [2, 4, 8, 16, 32]:
                    dst = dst_list[which]; which ^= 1
                    s4 = src.rearrange("p h (nb k) -> p h nb k", k=k)
                    d4 = dst.rearrange("p h (nb k) -> p h nb k", k=k)

                    def flip_max(d4=d4, s4=s4, k=k):
                        nc.vector.tensor_tensor(
                            d4[:, :, :, 0:k // 2], s4[:, :, :, 0:k // 2],
                            s4[:, :, :, k - 1:k // 2 - 1:-1], op=Alu.max)

                    def flip_min(d4=d4, s4=s4, k=k):
                        nc.vector.tensor_tensor(
                            d4[:, :, :, k // 2:k], s4[:, :, :, k // 2:k],
                            s4[:, :, :, k // 2 - 1::-1], op=Alu.min)
                    ths += [flip_max, flip_min]
                    src = dst
                    d = k // 4
                    while d >= 1:
                        dst = dst_list[which]; which ^= 1
                        s5 = src.rearrange("p h (nb two d) -> p h nb two d",
                                           two=2, d=d)
                        d5 = dst.rearrange("p h (nb two d) -> p h nb two d",
                                           two=2, d=d)

                        def plain_max(d5=d5, s5=s5):
                            nc.vector.tensor_tensor(
                                d5[:, :, :, 0, :], s5[:, :, :, 0, :],
                                s5[:, :, :, 1, :], op=Alu.max)

                        def plain_min(d5=d5, s5=s5):
                            nc.vector.tensor_tensor(
                                d5[:, :, :, 1, :], s5[:, :, :, 0, :],
                                s5[:, :, :, 1, :], op=Alu.min)
                        ths += [plain_max, plain_min]
                        src = dst
                        d //= 2

                sg = src.rearrange("p (g h) m -> p g h m", g=2)

                def merge(sg=sg):
                    nc.vector.tensor_tensor(top32[:, g0:g0 + 2, :],
                                            sg[:, :, 0, :],
                                            sg[:, :, 1, ::-1], op=Alu.max)
                ths.append(merge)

                for g in (g0, g0 + 1):
                    def bns(g=g):
                        nc.vector.bn_stats(bnst[:, g, :], top32[:, g, :])

                    def bna(g=g):
                        nc.vector.bn_aggr(mv[:, g, :], bnst[:, g:g + 1, :])
                    ths += [bns, bna]

                def mu_op():
                    nc.vector.tensor_scalar(mu[:, g0:g0 + 2],
                                            mv[:, g0:g0 + 2, 0],
                                            scalar1=b3s, scalar2=None,
                                            op0=Alu.add)
                ths.append(mu_op)

                if do_sample:
                    def sqrt_op():
                        nc.scalar.activation(std[:, g0:g0 + 2],
                                             mv[:, g0:g0 + 2, 1], Act.Sqrt,
                                             scale=float(NTOP) / (NTOP - 1))
                    ths.append(sqrt_op)

                    epsv = eps_sbs[pair].rearrange("p (g m) -> p g m", m=M)
                    stdb = std[:, g0:g0 + 2].unsqueeze(2).to_broadcast(
                        (P, 2, M))
                    mub = mu[:, g0:g0 + 2].unsqueeze(2).to_broadcast((P, 2, M))

                    def smul(epsv=epsv, stdb=stdb):
                        nc.vector.tensor_tensor(tmp_s[:, g0:g0 + 2, :], epsv,
                                                stdb, op=Alu.mult)
                    ths.append(smul)

                    def sadd(mub=mub):
                        nc.vector.tensor_tensor(a_bm[:, g0:g0 + 2, :],
                                                tmp_s[:, g0:g0 + 2, :], mub,
                                                op=Alu.add)
                    ths.append(sadd)

                    for j in range(2):
                        def aflip(j=j):
                            g = g0 + j
                            nc.sync.dma_start(
                                out=x[2:3, g * NPG:(g + 1) * NPG].rearrange(
                                    "a (p m) -> a p m", m=M),
                                in_=a_bm[:, g, :])
                        ths.append(aflip)
                return ths

            def phase(mlp_pair, tail):
                """Emit one pair's MLP with the other pair's tail interleaved
                into the vector/scalar/DMA instruction streams."""
                thunks = tail_thunks(*tail) if tail is not None else []
                if mlp_pair is None:
                    for th in thunks:
                        th()
                    return
                gen = mlp_tile_gen(mlp_pair)
                nt = 2 * len(tiles)
                per = (len(thunks) + nt - 1) // nt if thunks else 0
                ti = 0
                for _ in gen:
                    ti += 1
                    lo = per * (ti - 1)
                    for th in thunks[lo:lo + per]:
                        th()
                for th in thunks[per * ti:]:
                    th()

            # ---- prologue: t = 0 ----
            nc.sync.dma_start(out=x[2:3, :],
                              in_=A0.ap().rearrange("(a n) -> a n", a=1))
            phase(0, None)
            if n_steps > 1:
                load_eps(0, 0)
            phase(1, (0, n_steps > 1))

            # ---- t = 1 .. n_steps-2: pipelined loop ----
            if n_steps > 2:
                with tc.For_i(1, n_steps - 1,
                              hint_engines=(mybir.EngineType.PE,)) as it:
                    load_eps(1, it - 1)
                    load_eps(0, it)
                    phase(0, (1, True))   # pair B tail from iteration it-1
                    phase(1, (0, True))   # pair A tail of iteration it

            # ---- epilogue: t = n_steps-1 ----
            if n_steps > 1:
                load_eps(1, n_steps - 2)
                phase(0, (1, True))       # pair B tail from t = n_steps-2
                phase(1, (0, False))      # pair A final stats
            phase(None, (1, False))       # pair B final stats
            nc.vector.tensor_scalar(out_sb, mu, scalar1=TWO_PI, scalar2=None,
                                    op0=Alu.mult)
            nc.sync.dma_start(out=OUT.ap().rearrange("(g p) -> p g", p=P),
                              in_=out_sb)

    nc.compile()
    return nc


def host_prng(n_steps=ITERS - 1):
    """Exactly the reference's PRNG stream, on host CPU."""
    import jax
    import jax.numpy as jnp
    cpu = jax.devices("cpu")[0]
    with jax.default_device(cpu):
        # commit the key to CPU so the whole PRNG stream is computed by the
        # CPU backend bit-exactly (matching the reference harness)
        key = jax.device_put(jax.random.key(42), cpu)
        k0, kloop = jax.random.split(key)
        angles0 = np.asarray(jax.random.uniform(k0, (BATCH, M),
                                                dtype=jnp.float32))
        keys = jax.random.split(kloop, ITERS - 1)
        eps = np.stack([
            np.asarray(jax.random.normal(keys[t], (BATCH, M),
                                         dtype=jnp.float32))
            for t in range(max(n_steps - 1, 1))
        ])
    return angles0, eps


def make_in_map(core, states, W1, b1, W2, b2, W3, b3, angles0, eps):
    sl = slice(core * B, (core + 1) * B)
    S = np.ascontiguousarray(states[sl]).reshape(G, P, 2)
    xrep = np.ascontiguousarray(
        np.broadcast_to(S[:, :, None, :], (G, P, M, 2)).transpose(3, 0, 1, 2)
    ).reshape(2, N)
    a0 = np.ascontiguousarray(angles0[sl]).reshape(N)
    epsc = np.ascontiguousarray(
        eps[:, sl, :].reshape(-1, G, P, M).transpose(0, 2, 1, 3)
    ).reshape(-1, P, G * M)
    w3p = np.zeros((HIDDEN, 64), np.float16)
    w3p[:, 0] = W3[:, 0].astype(np.float16)
    return {
        "XREP": xrep.astype(np.float32),
        "A0": a0.astype(np.float32),
        "EPS": epsc.astype(np.float32),
        "W1D": W1.astype(np.float32),
        "W2D": W2.astype(np.float32),
        "W3D": w3p,
        "B1D": b1.astype(np.float32),
        "B2D": b2.astype(np.float32),
        "B3D": np.full((P,), np.float32(b3[0]), np.float32),
    }


LAST_RESULTS = None


def kernel(states, W1, b1, W2, b2, W3, b3, _trace=False):
    global LAST_RESULTS
    from concourse.bass_utils import run_bass_kernel_spmd

    n_steps = ITERS - 1
    if n_steps not in _PROG_CACHE:
        _PROG_CACHE[n_steps] = build_program(n_steps)
    nc = _PROG_CACHE[n_steps]

    angles0, eps = host_prng(n_steps)
    in_maps = [
        make_in_map(c, states, W1, b1, W2, b2, W3, b3, angles0, eps)
        for c in range(NCORES)
    ]
    res = run_bass_kernel_spmd(nc, in_maps, core_ids=list(range(NCORES)),
                               trace=_trace)
    LAST_RESULTS = res
    out = np.concatenate([res.results[c]["OUT"] for c in range(NCORES)])
    return out.astype(np.float32)
